# revision 14
# baseline (speedup 1.0000x reference)
"""Trainium2 Bass kernel for nn_CGPT_59725815218914 (dense transformer,
trunk/branch linear-attention model, B=4, T1=16384, T2=2048, H=256).

Sharding: 8 cores = 4 batch elements x 2 T1-halves. Each core owns 8192 T1
tokens of one batch element. Branch/z work is duplicated within each pair;
the self-attention k^T v context (tiny, [8,32,33] per head set) is
AllReduce'd across the pair.

On-chip layout: residual stream x is SBUF-resident as [128, NT*256] f32
(partition = token mod 128, col = tile*256 + channel). Matmuls run in fp32r
(TF32-class, full PE rate at N>=256). Linears compute out[tok,fout] =
(x^T).T @ W with PE-transposes producing the lhsT operands. All LN affine
params and biases are folded into adjacent matmuls host-side.
"""
import contextlib
import time

import numpy as np

import concourse.bass as bass
import concourse.mybir as mybir
import concourse.tile as tile
import concourse.bacc as bacc

F32 = mybir.dt.float32
F32R = mybir.dt.float32r
AF = mybir.ActivationFunctionType
ALU = mybir.AluOpType

H = 256
NHEAD = 8
DH = 32
NINNER = 1024
B = 4
T1 = 16384
T2 = 2048
NCORES = 8
EPS = 1e-5

NT_FULL = (T1 // 2) // 128     # 64 x-tiles per core
NZT_FULL = T2 // 128           # 16 z-tiles per core
NB_FULL = 3

AUX = H + NHEAD                # ctx psum cols: 256 ctx + 8 ksum
BDC = H + 2 * NHEAD            # cross bd cols: + 8 Z-ones cols


# ----------------------------------------------------------------------------
# host-side weight prep
# ----------------------------------------------------------------------------

def _np(a):
    return np.asarray(a, dtype=np.float32)


def _fold_ln(g, b, W, bias):
    return (g[:, None] * W).astype(np.float32), (b @ W + bias).astype(np.float32)


def build_weight_arrays(params, nblocks):
    out = {}

    def lin(prefix, W, b):
        out[prefix + "_w"] = _np(W)
        out[prefix + "_b"] = _np(b).reshape(1, -1)

    def mlp(prefix, p):
        lin(prefix + "_pre", p["pre"]["W"], p["pre"]["b"])
        for i, lp in enumerate(p["mid"]):
            lin(f"{prefix}_m{i}", lp["W"], lp["b"])
        lin(prefix + "_post", p["post"]["W"], p["post"]["b"])

    mlp("tr", params["trunk"])
    mlp("br", params["branch"][0])
    mlp("out", params["out"])
    Wop = np.zeros((H, 4), np.float32)
    Wop[:, 0:3] = out["out_post_w"]
    out["out_post_w"] = Wop
    bop = np.zeros((1, 4), np.float32)
    bop[:, 0:3] = out["out_post_b"]
    out["out_post_b"] = bop

    for bl in range(nblocks):
        p = params["blocks"][bl]
        g1, b1 = _np(p["ln1"]["g"]), _np(p["ln1"]["b"])
        g2, b2 = _np(p["ln2"][0]["g"]), _np(p["ln2"][0]["b"])
        g3, b3 = _np(p["ln3"]["g"]), _np(p["ln3"]["b"])
        g4, b4 = _np(p["ln4"]["g"]), _np(p["ln4"]["b"])
        g5, b5 = _np(p["ln5"]["g"]), _np(p["ln5"]["b"])

        c = p["cross"]
        lin(f"b{bl}_cq", *_fold_ln(g1, b1, _np(c["q"]["W"]), _np(c["q"]["b"])))
        lin(f"b{bl}_ck", *_fold_ln(g2, b2, _np(c["k"][0]["W"]), _np(c["k"][0]["b"])))
        lin(f"b{bl}_cv", *_fold_ln(g2, b2, _np(c["v"][0]["W"]), _np(c["v"][0]["b"])))
        lin(f"b{bl}_cp", _np(c["proj"]["W"]), _np(c["proj"]["b"]))

        s = p["self"]
        lin(f"b{bl}_sq", *_fold_ln(g4, b4, _np(s["q"]["W"]), _np(s["q"]["b"])))
        lin(f"b{bl}_sk", *_fold_ln(g4, b4, _np(s["k"][0]["W"]), _np(s["k"][0]["b"])))
        lin(f"b{bl}_sv", *_fold_ln(g4, b4, _np(s["v"][0]["W"]), _np(s["v"][0]["b"])))
        lin(f"b{bl}_sp", _np(s["proj"]["W"]), _np(s["proj"]["b"]))

        for tag, (gg, bb), fp in (("f1", (g3, b3), p["mlp1"]),
                                  ("f2", (g5, b5), p["mlp2"])):
            W1, bf1 = _fold_ln(gg, bb, _np(fp["fc1"]["W"]), _np(fp["fc1"]["b"]))
            out[f"b{bl}_{tag}_w1"] = W1
            out[f"b{bl}_{tag}_b1"] = bf1.reshape(NINNER // 128, 128).T.copy()
            lin(f"b{bl}_{tag}_fc2", _np(fp["fc2"]["W"]), _np(fp["fc2"]["b"]))

    out["ident"] = np.eye(128, dtype=np.float32)
    out["bdzero"] = np.zeros((128, H + 2 * NHEAD), dtype=np.float32)
    out["ones_row"] = np.ones((1, 128), dtype=np.float32)
    zsel = np.zeros((H, NHEAD), dtype=np.float32)
    for h in range(NHEAD):
        zsel[h * DH:(h + 1) * DH, h] = 1.0
    out["zsel"] = zsel
    return out


# ----------------------------------------------------------------------------
# device program
# ----------------------------------------------------------------------------

class Prog:
    def __init__(self, nc, tc, nt, nzt, nblocks, use_cc):
        self.nc, self.tc = nc, tc
        self.nt, self.nzt, self.nblocks, self.use_cc = nt, nzt, nblocks, use_cc
        self.input_names = []

    def dram_in(self, name, shape, dt):
        ap = self.nc.dram_tensor(name, shape, dt, kind="ExternalInput").ap()
        self.input_names.append(name)
        return ap

    def build(self):
        nc, tc = self.nc, self.tc
        nt, nzt = self.nt, self.nzt

        self.d_x = self.dram_in("x_in", (nt * 128, 3), F32)
        self.d_y0 = self.dram_in("y0_in", (nzt * 128, 3), F32)
        self.d_out = nc.dram_tensor("y_out", (nt * 128, 3), F32,
                                    kind="ExternalOutput").ap()

        # weight dram tensors
        wnames = []
        for pre in ("tr", "br", "out"):
            for part in ("pre", "m0", "m1", "post"):
                wnames += [f"{pre}_{part}_w", f"{pre}_{part}_b"]
        for bl in range(self.nblocks):
            for t in ("cq", "ck", "cv", "cp", "sq", "sk", "sv", "sp"):
                wnames += [f"b{bl}_{t}_w", f"b{bl}_{t}_b"]
            for t in ("f1", "f2"):
                wnames += [f"b{bl}_{t}_w1", f"b{bl}_{t}_b1",
                           f"b{bl}_{t}_fc2_w", f"b{bl}_{t}_fc2_b"]
        wnames += ["ident", "ones_row", "zsel", "bdzero"]

        def wshape(n):
            if n == "ident":
                return (128, 128)
            if n == "ones_row":
                return (1, 128)
            if n == "zsel":
                return (H, NHEAD)
            if n == "bdzero":
                return (128, BDC)
            if n.endswith("_w1"):
                return (H, NINNER)
            if n.endswith("_b1"):
                return (128, NINNER // 128)
            if n.endswith("fc2_w"):
                return (NINNER, H)
            if n in ("tr_pre_w", "br_pre_w"):
                return (3, H)
            if n == "out_post_w":
                return (H, 4)
            if n == "out_post_b":
                return (1, 4)
            if n.endswith("_b"):
                return (1, H)
            return (H, H)

        self.d_w = {n: self.dram_in(n, wshape(n), F32 if n.endswith("_b1") else F32R)
                    for n in wnames}

        st = contextlib.ExitStack()
        self.res = st.enter_context(tc.tile_pool(name="res", bufs=1))
        self.wp = st.enter_context(tc.tile_pool(name="wp", bufs=2))
        self.wk = st.enter_context(tc.tile_pool(name="wkonst", bufs=1))
        self.work = st.enter_context(tc.tile_pool(name="work", bufs=3))
        self.small = st.enter_context(tc.tile_pool(name="small", bufs=6))
        self.ps_mm = st.enter_context(tc.tile_pool(name="psmm", bufs=3, space="PSUM"))
        self.ps_tp = st.enter_context(tc.tile_pool(name="pstp", bufs=3, space="PSUM"))
        self.ps_ctx = st.enter_context(tc.tile_pool(name="psctx", bufs=1, space="PSUM"))
        self.dram = st.enter_context(tc.tile_pool(name="dramp", bufs=1, space="DRAM"))

        # constants
        self.ident = self.wk.tile([128, 128], F32R, tag="ident")
        nc.sync.dma_start(self.ident[:], self.d_w["ident"])
        self.ones = self.wk.tile([1, 128], F32R, tag="ones")
        nc.sync.dma_start(self.ones[:], self.d_w["ones_row"])
        self.zsel = self.wk.tile([128, 2 * NHEAD], F32R, tag="zsel")
        nc.sync.dma_start(
            self.zsel[:].rearrange("p (k n) -> p k n", k=2),
            self.d_w["zsel"].rearrange("(k p) n -> p k n", p=128))
        self.bdzero = self.wk.tile([128, BDC], F32R, tag="bdzero")
        nc.sync.dma_start(self.bdzero[:], self.d_w["bdzero"])

        # residents
        self.x_res = self.res.tile([128, nt * H], F32, tag="x_res")
        self.z0T = self.res.tile([128, nzt * H], F32R, tag="z0T")
        self.xin = self.res.tile([128, nt * 3], F32, tag="xin")
        self.yout = self.res.tile([128, nt * 4], F32, tag="yout")
        self.yin = self.res.tile([128, nzt * 3], F32, tag="yin")
        nc.sync.dma_start(
            self.xin[:].rearrange("p (i c) -> p i c", c=3),
            self.d_x.rearrange("(i p) c -> p i c", p=128))
        nc.sync.dma_start(
            self.yin[:].rearrange("p (i c) -> p i c", c=3),
            self.d_y0.rearrange("(i p) c -> p i c", p=128))

        self.q_dram = self.dram.tile([128, nt * H], F32R, tag="q_dram")
        if self.use_cc:
            self.ar_in = self.dram.tile([128, 2 * AUX], F32, tag="ar_in")
            self.ar_out = self.dram.tile([128, 2 * AUX], F32, tag="ar_out")

        # ---- program ----
        # branch first: z scratch borrows the (not yet written) x_res cols
        self.branch_pass()
        self.mlp_pass(self.xin, nt, "tr", dst=self.x_res)
        for bl in range(self.nblocks):
            bd_c = self.cross_ctx(bl)
            self.attn_qside(bl, "c", bd_c)
            self.ffn(bl, "f1")
            self.self_attn(bl)
            self.ffn(bl, "f2")
        self.mlp_pass(self.x_res, nt, "out", dst=self.yout)
        nc.sync.dma_start(
            self.d_out.rearrange("(i p) c -> p i c", p=128),
            self.yout[:].rearrange("p (i c) -> p i c", c=4)[:, :, 0:3])
        st.close()

    # ---- primitives ----------------------------------------------------
    def load_w(self, name, tag):
        """[fin, fout] dram -> sbuf tile [P, kt*fout] (k-major col blocks)."""
        nc = self.nc
        d = self.d_w[name]
        fin, fout = d.shape
        if fin > 128:
            kt = fin // 128
            t = self.wp.tile([128, kt * fout], F32R, tag=tag,
                 bufs=1 if tag in ("w1", "wf2") else None)
            nc.sync.dma_start(
                t[:].rearrange("p (k n) -> p k n", k=kt),
                d.rearrange("(k p) n -> p k n", p=128))
        else:
            kt = 1
            t = self.wp.tile([fin, fout], F32R, tag=tag)
            nc.sync.dma_start(t[:], d)
        return t, kt, fout

    def load_brow(self, name):
        nc = self.nc
        d = self.d_w[name]
        t = self.small.tile([1, H], F32R, tag="brow")
        nc.sync.dma_start(t[0:1, 0:d.shape[1]], d)
        return t

    def transpose(self, src_ap, n_ch, dst_ap=None, in_f32=False, tag="Tt"):
        """[128, n_ch] sbuf -> transposed [128, n_ch] (k-major 128-blocks)."""
        nc = self.nc
        kt = n_ch // 128
        ps = self.ps_tp.tile([128, 256], F32 if in_f32 else F32R, tag="tp")
        ident = self.ident[:].bitcast(F32) if in_f32 else self.ident[:]
        for k in range(kt):
            nc.tensor.matmul(ps[:, k * 128:(k + 1) * 128],
                             src_ap[:, k * 128:(k + 1) * 128],
                             ident, is_transpose=True)
        if dst_ap is None:
            t = self.work.tile([128, n_ch], F32R, tag=tag)
            dst_ap = t[:]
        else:
            t = None
        nc.scalar.copy(dst_ap, ps[:, 0:n_ch])
        return t

    def linear(self, lhsT_aps, rhs_aps, n, brow=None, psum=None, tag="mm"):
        nc = self.nc
        if psum is None:
            psum = self.ps_mm.tile([128, AUX + NHEAD], F32, tag=tag)
        o = psum[:, 0:n]
        first = True
        if brow is not None:
            nc.tensor.matmul(o, self.ones[:], brow[0:1, 0:n], start=True,
                             stop=False)
            first = False
        last = len(lhsT_aps) - 1
        for k, (lt, rt) in enumerate(zip(lhsT_aps, rhs_aps)):
            nc.tensor.matmul(o, lt, rt, start=first, stop=(k == last))
            first = False
        return psum

    def wslices(self, W, kt, n):
        return [W[:, k * n:(k + 1) * n] for k in range(kt)]

    def tslices(self, xT, kt=2):
        return [xT[:, k * 128:(k + 1) * 128] for k in range(kt)]

    def ln_stats(self, x_slice_3d, ntile):
        nc = self.nc
        st = self.small.tile([128, ntile * 6], F32, tag="st")
        mv = self.small.tile([128, ntile * 2], F32, tag="mv")
        for t in range(ntile):
            nc.vector.bn_stats(st[:, 6 * t:6 * t + 6], x_slice_3d[:, t])
            nc.vector.bn_aggr(mv[:, 2 * t:2 * t + 2], st[:, 6 * t:6 * t + 6])
        ve = self.small.tile([128, ntile], F32, tag="ve")
        mv3 = mv[:].rearrange("p (t s) -> p t s", s=2)
        nc.vector.tensor_scalar(ve[:].rearrange("p (t s) -> p t s", s=1),
                                mv3[:, :, 1:2], EPS, None, ALU.add)
        nc.scalar.activation(ve[:], ve[:], AF.Sqrt)
        nc.vector.reciprocal(ve[:], ve[:])
        return mv, ve

    def ln_norm(self, x_slice, mv, ve, t):
        nc = self.nc
        xh = self.work.tile([128, H], F32R, tag="xhat")
        nc.vector.tensor_scalar(xh[:], x_slice, mv[:, 2 * t:2 * t + 1],
                                ve[:, t:t + 1], ALU.subtract, ALU.mult)
        return xh

    # ---- phases --------------------------------------------------------
    def mlp_pass(self, src, ntiles, pre, dst):
        """dst tile cols = ntiles*fout. src: [128, ntiles*3] or x_res."""
        nc = self.nc
        in3 = pre != "out"
        Wp, ktp, foutp = self.load_w(pre + "_pre_w", tag="wA")
        bp = self.load_brow(pre + "_pre_b")
        W0, kt0, _ = self.load_w(pre + "_m0_w", tag="wB")
        b0 = self.load_brow(pre + "_m0_b")
        W1, kt1, _ = self.load_w(pre + "_m1_w", tag="wC")
        b1 = self.load_brow(pre + "_m1_b")
        Wq, ktq, foutq = self.load_w(pre + "_post_w", tag="wD")
        bq = self.load_brow(pre + "_post_b")

        for i in range(ntiles):
            if in3:
                ps = self.ps_tp.tile([128, 256], F32, tag="tp")
                nc.tensor.matmul(ps[0:3, 0:128], src[:, i * 3:(i + 1) * 3],
                                 self.ident[:].bitcast(F32), is_transpose=True)
                xT = self.work.tile([3, 128], F32R, tag="Tin3", bufs=2)
                nc.scalar.copy(xT[:], ps[0:3, 0:128])
                lhs = [xT[:]]
            else:
                xT = self.transpose(src[:, i * H:(i + 1) * H], H, in_f32=True, tag='xT')
                lhs = self.tslices(xT[:])
            p = self.linear(lhs, self.wslices(Wp, ktp, foutp), foutp, brow=bp)
            g1 = self.work.tile([128, H], F32R, tag="mlpg", bufs=2)
            nc.scalar.activation(g1[:], p[:, 0:H], AF.Gelu)
            cur = g1
            for W, bb, kt in ((W0, b0, kt0), (W1, b1, kt1)):
                cT = self.transpose(cur[:], H, tag='cT')
                p = self.linear(self.tslices(cT[:]), self.wslices(W, kt, H), H,
                                brow=bb)
                nxt = self.work.tile([128, H], F32R, tag="mlpg", bufs=2)
                nc.scalar.activation(nxt[:], p[:, 0:H], AF.Gelu)
                nc.vector.tensor_tensor(nxt[:], nxt[:], cur[:], ALU.add)
                cur = nxt
            cT = self.transpose(cur[:], H, tag='cT')
            p = self.linear(self.tslices(cT[:]), self.wslices(Wq, ktq, foutq),
                            foutq, brow=bq)
            nc.scalar.copy(dst[:, i * foutq:(i + 1) * foutq], p[:, 0:foutq])

    def branch_pass(self):
        nc = self.nc
        z_tmp = self.x_res  # borrow before trunk writes it
        self.mlp_pass(self.yin, self.nzt, "br", dst=z_tmp)
        for i0 in range(0, self.nzt, 2):
            ntile = min(2, self.nzt - i0)
            x3 = z_tmp[:, i0 * H:(i0 + ntile) * H].rearrange(
                "p (t c) -> p t c", c=H)
            mv, ve = self.ln_stats(x3, ntile)
            for t in range(ntile):
                i = i0 + t
                zh = self.ln_norm(z_tmp[:, i * H:(i + 1) * H], mv, ve, t)
                self.transpose(zh[:], H, dst_ap=self.z0T[:, i * H:(i + 1) * H])

    def kv_ctx(self, lhsT_aps, Wk, bk, Wv, bv, ctx, first, last):
        """exp-k / Zk-folded-v projections of one tile; rank-update ctx."""
        nc = self.nc
        kp = self.linear(lhsT_aps, self.wslices(Wk, 2, H), H, brow=bk)
        ek = self.work.tile([128, H], F32R, tag="ek")
        nc.scalar.activation(ek[:], kp[:, 0:H], AF.Exp)
        ekT = self.transpose(ek[:], H, tag='ekT')
        zp = self.linear(self.tslices(ekT[:]), self.wslices(self.zsel, 2, NHEAD),
                         NHEAD)
        va = self.work.tile([128, AUX], F32R, tag="vaug")
        with nc.allow_low_precision(reason="f32r rounding of 1/Zk is fine"):
            nc.vector.reciprocal(va[:, H:AUX], zp[:, 0:NHEAD])
        vp = self.linear(lhsT_aps, self.wslices(Wv, 2, H), H, brow=bv)
        nc.vector.tensor_tensor(
            va[:, 0:H].rearrange("p (g c) -> p g c", g=NHEAD),
            vp[:, 0:H].rearrange("p (g c) -> p g c", g=NHEAD),
            va[:, H:AUX].to_broadcast((128, NHEAD, DH)),
            ALU.mult)
        for m in range(2):
            nc.tensor.matmul(ctx[m][:, 0:AUX], ek[:, m * 128:(m + 1) * 128],
                             va[:, 0:AUX], start=first, stop=last)

    def build_bd(self, src, ncols, z_ones):
        """src(k) -> [128, AUX] AP. Returns bd tiles [2][128, ncols]."""
        nc = self.nc
        bd = []
        for k in range(2):
            t = self.work.tile([128, ncols], F32R, tag="bd", bufs=2)
            nc.vector.tensor_copy(t[:], self.bdzero[:, 0:ncols])
            s = src(k)
            for j in range(4):
                h = 4 * k + j
                r0, r1 = 32 * j, 32 * j + 32
                nc.vector.tensor_copy(t[r0:r1, h * DH:(h + 1) * DH],
                                      s[r0:r1, h * DH:(h + 1) * DH])
                nc.vector.tensor_copy(t[r0:r1, H + h:H + h + 1],
                                      s[r0:r1, H + h:H + h + 1])
            if z_ones:
                nc.vector.tensor_copy(t[:, H + NHEAD:H + 2 * NHEAD],
                                      self.zsel[:, k * NHEAD:(k + 1) * NHEAD])
            bd.append(t)
        return bd

    def cross_ctx(self, bl):
        nc = self.nc
        Wk, _, _ = self.load_w(f"b{bl}_ck_w", tag="wB")
        bk = self.load_brow(f"b{bl}_ck_b")
        Wv, _, _ = self.load_w(f"b{bl}_cv_w", tag="wC")
        bv = self.load_brow(f"b{bl}_cv_b")
        ctx = [self.ps_ctx.tile([128, AUX], F32, tag=f"ctx{m}", name=f"ctx{m}")
               for m in range(2)]
        for i in range(self.nzt):
            self.kv_ctx(self.tslices(self.z0T[:, i * H:(i + 1) * H]),
                        Wk, bk, Wv, bv, ctx, i == 0, i == self.nzt - 1)
        return self.build_bd(lambda k: ctx[k][:, 0:AUX], BDC, z_ones=True)

    def attn_qside(self, bl, which, bd):
        """Fused q-side attention + proj + residual (cross: which='c')."""
        nc = self.nc
        Wq, _, _ = self.load_w(f"b{bl}_{which}q_w", tag="wA")
        bq = self.load_brow(f"b{bl}_{which}q_b")
        Wp, _, _ = self.load_w(f"b{bl}_{which}p_w", tag="wD")
        bp = self.load_brow(f"b{bl}_{which}p_b")
        for i0 in range(0, self.nt, 2):
            x3 = self.x_res[:, i0 * H:(i0 + 2) * H].rearrange(
                "p (t c) -> p t c", c=H)
            mv, ve = self.ln_stats(x3, 2)
            for t in range(2):
                i = i0 + t
                xh = self.ln_norm(self.x_res[:, i * H:(i + 1) * H], mv, ve, t)
                xT = self.transpose(xh[:], H, tag='xT')
                qp = self.linear(self.tslices(xT[:]), self.wslices(Wq, 2, H),
                                 H, brow=bq)
                qe = self.work.tile([128, H], F32R, tag="qe")
                nc.scalar.activation(qe[:], qp[:, 0:H], AF.Exp)
                qT = self.transpose(qe[:], H, tag='qT')
                npp = self.linear(self.tslices(qT[:]),
                                  [bd[0][:], bd[1][:]], BDC)
                cb = self.small.tile([128, 2 * NHEAD], F32, tag="cb")
                nc.vector.reciprocal(cb[:], npp[:, H:H + 2 * NHEAD])
                t1 = self.work.tile([128, H], F32R, tag="t1")
                nc.vector.tensor_tensor(
                    t1[:].rearrange("p (g c) -> p g c", g=NHEAD),
                    npp[:, 0:H].rearrange("p (g c) -> p g c", g=NHEAD),
                    cb[:, 0:NHEAD].to_broadcast((128, NHEAD, DH)), ALU.mult)
                qx = self.work.tile([128, H], F32R, tag="qx", bufs=2)
                nc.vector.tensor_tensor(
                    qx[:].rearrange("p (g c) -> p g c", g=NHEAD),
                    qe[:].rearrange("p (g c) -> p g c", g=NHEAD),
                    cb[:, NHEAD:2 * NHEAD].to_broadcast((128, NHEAD, DH)),
                    ALU.mult)
                nc.vector.tensor_tensor(t1[:], t1[:], qx[:], ALU.add)
                self.attn_tail(t1, Wp, bp, i)

    def attn_tail(self, attn, Wp, bp, i):
        nc = self.nc
        aT = self.transpose(attn[:], H, tag='aT')
        pp = self.linear(self.tslices(aT[:]), self.wslices(Wp, 2, H), H,
                         brow=bp)
        xs = self.x_res[:, i * H:(i + 1) * H]
        nc.vector.tensor_tensor(xs, xs, pp[:, 0:H], ALU.add)

    def self_attn(self, bl):
        nc = self.nc
        Wq, _, _ = self.load_w(f"b{bl}_sq_w", tag="wA")
        bq = self.load_brow(f"b{bl}_sq_b")
        Wk, _, _ = self.load_w(f"b{bl}_sk_w", tag="wB")
        bk = self.load_brow(f"b{bl}_sk_b")
        Wv, _, _ = self.load_w(f"b{bl}_sv_w", tag="wC")
        bv = self.load_brow(f"b{bl}_sv_b")
        ctx = [self.ps_ctx.tile([128, AUX], F32, tag=f"ctx{m}", name=f"ctx{m}")
               for m in range(2)]
        # pass 1: q-> normalize -> q_dram; k/v -> ctx
        for i0 in range(0, self.nt, 2):
            x3 = self.x_res[:, i0 * H:(i0 + 2) * H].rearrange(
                "p (t c) -> p t c", c=H)
            mv, ve = self.ln_stats(x3, 2)
            for t in range(2):
                i = i0 + t
                xh = self.ln_norm(self.x_res[:, i * H:(i + 1) * H], mv, ve, t)
                xT = self.transpose(xh[:], H)
                lhs = self.tslices(xT[:])
                qp = self.linear(lhs, self.wslices(Wq, 2, H), H, brow=bq)
                qe = self.work.tile([128, H], F32R, tag="qe")
                nc.scalar.activation(qe[:], qp[:, 0:H], AF.Exp)
                qT = self.transpose(qe[:], H, tag='qT')
                zq = self.linear(self.tslices(qT[:]),
                                 self.wslices(self.zsel, 2, NHEAD), NHEAD)
                zi = self.small.tile([128, NHEAD], F32, tag="zi")
                nc.vector.reciprocal(zi[:], zq[:, 0:NHEAD])
                qn = self.work.tile([128, H], F32R, tag="qn")
                nc.vector.tensor_tensor(
                    qn[:].rearrange("p (g c) -> p g c", g=NHEAD),
                    qe[:].rearrange("p (g c) -> p g c", g=NHEAD),
                    zi[:].to_broadcast((128, NHEAD, DH)), ALU.mult)
                nc.sync.dma_start(self.q_dram[:, i * H:(i + 1) * H], qn[:])
                self.kv_ctx(lhs, Wk, bk, Wv, bv, ctx, i == 0, i == self.nt - 1)

        # combine pair ctx
        if self.use_cc:
            stage = self.work.tile([128, 2 * AUX], F32, tag="stage", bufs=1)
            for k in range(2):
                nc.scalar.copy(stage[:, k * AUX:(k + 1) * AUX], ctx[k][:, 0:AUX])
            nc.sync.dma_start(self.ar_in[:], stage[:])
            nc.gpsimd.collective_compute(
                "AllReduce", ALU.add,
                replica_groups=[[0, 1], [2, 3], [4, 5], [6, 7]],
                ins=[self.ar_in.opt()], outs=[self.ar_out.opt()])
            stage_r = self.work.tile([128, 2 * AUX], F32, tag="stager", bufs=1)
            nc.sync.dma_start(stage_r[:], self.ar_out[:])
            bd = self.build_bd(lambda k: stage_r[:, k * AUX:(k + 1) * AUX],
                               AUX, z_ones=False)
        else:
            bd = self.build_bd(lambda k: ctx[k][:, 0:AUX], AUX, z_ones=False)

        Wp, _, _ = self.load_w(f"b{bl}_sp_w", tag="wD")
        bp = self.load_brow(f"b{bl}_sp_b")
        # pass 2
        for i in range(self.nt):
            qn = self.work.tile([128, H], F32R, tag="qn")
            nc.sync.dma_start(qn[:], self.q_dram[:, i * H:(i + 1) * H])
            qT = self.transpose(qn[:], H, tag='qT')
            npp = self.linear(self.tslices(qT[:]), [bd[0][:], bd[1][:]], AUX)
            cb = self.small.tile([128, NHEAD], F32, tag="cb2")
            nc.vector.reciprocal(cb[:], npp[:, H:AUX])
            t1 = self.work.tile([128, H], F32R, tag="t1")
            nc.vector.tensor_tensor(
                t1[:].rearrange("p (g c) -> p g c", g=NHEAD),
                npp[:, 0:H].rearrange("p (g c) -> p g c", g=NHEAD),
                cb[:].to_broadcast((128, NHEAD, DH)), ALU.mult)
            nc.vector.tensor_tensor(t1[:], t1[:], qn[:], ALU.add)
            self.attn_tail(t1, Wp, bp, i)

    def ffn(self, bl, tag):
        nc = self.nc
        W1, _, _ = self.load_w(f"b{bl}_{tag}_w1", tag="w1")
        b1 = self.wp.tile([128, NINNER // 128], F32, tag="b1c")
        nc.sync.dma_start(b1[:], self.d_w[f"b{bl}_{tag}_b1"])
        W2, _, _ = self.load_w(f"b{bl}_{tag}_fc2_w", tag="wf2")
        b2 = self.load_brow(f"b{bl}_{tag}_fc2_b")
        MT = NINNER // 128
        for i0 in range(0, self.nt, 2):
            x3 = self.x_res[:, i0 * H:(i0 + 2) * H].rearrange(
                "p (t c) -> p t c", c=H)
            mv, ve = self.ln_stats(x3, 2)
            xTc = self.work.tile([128, 2 * H], F32R, tag="xTc", bufs=2)
            for t in range(2):
                xh = self.ln_norm(self.x_res[:, (i0 + t) * H:(i0 + t + 1) * H],
                                  mv, ve, t)
                self.transpose(xh[:], H, dst_ap=xTc[:, t * H:(t + 1) * H])
            # fc1 (mapping A): h1T[m] = W1[:,m].T @ xhatT ; gelu w/ bias
            h1 = self.work.tile([128, MT * 256], F32R, tag="h1c", bufs=2)
            x3c = xTc[:].rearrange("p (t c) -> p t c", c=H)
            for m in range(MT):
                pm = self.ps_mm.tile([128, AUX + NHEAD], F32, tag="mm")
                for k in range(2):
                    nc.tensor.matmul(
                        pm[:, 0:256],
                        W1[:, k * NINNER + m * 128:k * NINNER + (m + 1) * 128],
                        x3c[:, :, k * 128:(k + 1) * 128],
                        start=(k == 0), stop=(k == 1))
                nc.scalar.activation(h1[:, m * 256:(m + 1) * 256], pm[:, 0:256],
                                     AF.Gelu, bias=b1[:, m:m + 1])
            # fc2 (mapping B): per tile
            for t in range(2):
                lhs = [h1[:, k * 256 + t * 128:k * 256 + t * 128 + 128]
                       for k in range(MT)]
                pf = self.linear(lhs, self.wslices(W2, MT, H), H, brow=b2)
                xs = self.x_res[:, (i0 + t) * H:(i0 + t + 1) * H]
                nc.vector.tensor_tensor(xs, xs, pf[:, 0:H], ALU.add)


# ----------------------------------------------------------------------------
# build + run
# ----------------------------------------------------------------------------

_CACHE = {}


def build_nc(nt=NT_FULL, nzt=NZT_FULL, nblocks=NB_FULL, use_cc=True,
             num_devices=NCORES):
    nc = bacc.Bacc("TRN2", target_bir_lowering=False, debug=False,
                   num_devices=num_devices)
    with tile.TileContext(nc) as tc:
        prog = Prog(nc, tc, nt, nzt, nblocks, use_cc)
        prog.build()
    nc.compile()
    return nc, prog


def _shard_inputs(x, y0, warrs, nt, nzt, ncores):
    """Returns per-core input dicts."""
    x = np.asarray(x, np.float32)
    y0 = np.asarray(y0, np.float32)
    maps = []
    half = nt * 128
    for c in range(ncores):
        b, h = c // 2, c % 2
        m = dict(warrs)
        m["x_in"] = np.ascontiguousarray(x[b, h * half:(h + 1) * half, :])
        m["y0_in"] = np.ascontiguousarray(y0[b, :nzt * 128, :])
        maps.append(m)
    return maps


class Runner:
    """Persistent PJRT executor for one built Bass program (axon path).
    Mirrors bass2jax.run_bass_via_pjrt but keeps the jitted callable so
    repeated executions don't recompile."""

    def __init__(self, nc, n_cores=NCORES):
        import jax
        from jax.sharding import Mesh, PartitionSpec
        from jax.experimental.shard_map import shard_map
        import concourse.mybir as mybir_
        from concourse import bass2jax

        bass2jax.install_neuronx_cc_hook()
        self.nc = nc
        self.n_cores = n_cores
        partition_name = (nc.partition_id_tensor.name
                          if nc.partition_id_tensor else None)
        in_names, out_names, out_avals, zero_outs = [], [], [], []
        for alloc in nc.m.functions[0].allocations:
            if not isinstance(alloc, mybir_.MemoryLocationSet):
                continue
            name = alloc.memorylocations[0].name
            if alloc.kind == "ExternalInput":
                if name == partition_name:
                    continue
                in_names.append(name)
            elif alloc.kind == "ExternalOutput":
                shape = tuple(alloc.tensor_shape)
                dtype = mybir_.dt.np(alloc.dtype)
                out_names.append(name)
                out_avals.append(jax.core.ShapedArray(shape, dtype))
                zero_outs.append(np.zeros(shape, dtype))
        self.in_names, self.out_names = in_names, out_names
        self.out_avals, self.zero_outs = out_avals, zero_outs
        n_params, n_outs = len(in_names), len(out_names)
        all_in = in_names + out_names
        if partition_name is not None:
            all_in = all_in + [partition_name]

        def _body(*args):
            operands = list(args)
            if partition_name is not None:
                operands.append(bass2jax.partition_id_tensor())
            outs = bass2jax._bass_exec_p.bind(
                *operands,
                out_avals=tuple(out_avals),
                in_names=tuple(all_in),
                out_names=tuple(out_names),
                lowering_input_output_aliases=(),
                sim_require_finite=True,
                sim_require_nnan=True,
                nc=nc,
            )
            return tuple(outs)

        devices = jax.devices()[:n_cores]
        mesh = Mesh(np.asarray(devices), ("core",))
        donate = tuple(range(n_params, n_params + n_outs))
        self._fn = jax.jit(
            shard_map(_body, mesh=mesh,
                      in_specs=(PartitionSpec("core"),) * (n_params + n_outs),
                      out_specs=(PartitionSpec("core"),) * n_outs,
                      check_rep=False),
            donate_argnums=donate, keep_unused=True)

    def __call__(self, in_maps):
        concat_in = [
            np.concatenate([np.asarray(m[name]) for m in in_maps], axis=0)
            for name in self.in_names]
        concat_zeros = [
            np.zeros((self.n_cores * z.shape[0], *z.shape[1:]), z.dtype)
            for z in self.zero_outs]
        out_arrs = self._fn(*concat_in, *concat_zeros)
        return [
            {name: np.asarray(out_arrs[i]).reshape(
                self.n_cores, *self.out_avals[i].shape)[c]
             for i, name in enumerate(self.out_names)}
            for c in range(self.n_cores)]

    def run_async(self, in_maps):
        """Dispatch without converting outputs (returns jax arrays)."""
        concat_in = [
            np.concatenate([np.asarray(m[name]) for m in in_maps], axis=0)
            for name in self.in_names]
        concat_zeros = [
            np.zeros((self.n_cores * z.shape[0], *z.shape[1:]), z.dtype)
            for z in self.zero_outs]
        return self._fn(*concat_in, *concat_zeros)


def run_spmd(nc, in_maps):
    import concourse.bass_utils as bass_utils
    res = bass_utils.run_bass_kernel_spmd(
        nc, in_maps, core_ids=list(range(len(in_maps))))
    return res.results


def get_runner():
    if "full" not in _CACHE:
        nc, prog = build_nc()
        _CACHE["full"] = (nc, prog)
        _CACHE["runner"] = Runner(nc)
    return _CACHE["runner"]


def kernel(x, y0, params):
    x = np.asarray(x, np.float32)
    y0 = np.asarray(y0, np.float32)
    runner = get_runner()
    warrs = build_weight_arrays(params, NB_FULL)
    in_maps = _shard_inputs(x, y0, warrs, NT_FULL, NZT_FULL, NCORES)
    results = runner(in_maps)
    out = np.empty((B, T1, 3), np.float32)
    half = NT_FULL * 128
    for c in range(NCORES):
        b, h = c // 2, c % 2
        out[b, h * half:(h + 1) * half, :] = results[c]["y_out"]
    return out


# revision 15
# speedup vs baseline: 34.1121x; 34.1121x over previous
"""Trainium2 Bass kernel for nn_CGPT_59725815218914 (dense transformer,
trunk/branch linear-attention model, B=4, T1=16384, T2=2048, H=256).

Sharding: 8 cores = 4 batch elements x 2 T1-halves. Each core owns 8192 T1
tokens of one batch element. Branch/z work is duplicated within each pair;
the self-attention k^T v context (tiny, [8,32,33] per head set) is
AllReduce'd across the pair.

On-chip layout: residual stream x is SBUF-resident as [128, NT*256] f32
(partition = token mod 128, col = tile*256 + channel). Matmuls run in fp32r
(TF32-class, full PE rate at N>=256). Linears compute out[tok,fout] =
(x^T).T @ W with PE-transposes producing the lhsT operands. All LN affine
params and biases are folded into adjacent matmuls host-side.
"""
import contextlib
import time

import numpy as np

import concourse.bass as bass
import concourse.mybir as mybir
import concourse.tile as tile
import concourse.bacc as bacc

F32 = mybir.dt.float32
F32R = mybir.dt.float32r
AF = mybir.ActivationFunctionType
ALU = mybir.AluOpType

H = 256
NHEAD = 8
DH = 32
NINNER = 1024
B = 4
T1 = 16384
T2 = 2048
NCORES = 8
EPS = 1e-5

NT_FULL = (T1 // 2) // 128     # 64 x-tiles per core
NZT_FULL = T2 // 128           # 16 z-tiles per core
NB_FULL = 3

AUX = H + NHEAD                # ctx psum cols: 256 ctx + 8 ksum
BDC = H + 2 * NHEAD            # cross bd cols: + 8 Z-ones cols


# ----------------------------------------------------------------------------
# host-side weight prep
# ----------------------------------------------------------------------------

def _np(a):
    return np.asarray(a, dtype=np.float32)


def _fold_ln(g, b, W, bias):
    return (g[:, None] * W).astype(np.float32), (b @ W + bias).astype(np.float32)


def build_weight_arrays(params, nblocks):
    out = {}

    def lin(prefix, W, b):
        out[prefix + "_w"] = _np(W)
        out[prefix + "_b"] = _np(b).reshape(1, -1)

    def mlp(prefix, p):
        lin(prefix + "_pre", p["pre"]["W"], p["pre"]["b"])
        for i, lp in enumerate(p["mid"]):
            lin(f"{prefix}_m{i}", lp["W"], lp["b"])
        lin(prefix + "_post", p["post"]["W"], p["post"]["b"])

    mlp("tr", params["trunk"])
    mlp("br", params["branch"][0])
    mlp("out", params["out"])
    Wop = np.zeros((H, 4), np.float32)
    Wop[:, 0:3] = out["out_post_w"]
    out["out_post_w"] = Wop
    bop = np.zeros((1, 4), np.float32)
    bop[:, 0:3] = out["out_post_b"]
    out["out_post_b"] = bop

    for bl in range(nblocks):
        p = params["blocks"][bl]
        g1, b1 = _np(p["ln1"]["g"]), _np(p["ln1"]["b"])
        g2, b2 = _np(p["ln2"][0]["g"]), _np(p["ln2"][0]["b"])
        g3, b3 = _np(p["ln3"]["g"]), _np(p["ln3"]["b"])
        g4, b4 = _np(p["ln4"]["g"]), _np(p["ln4"]["b"])
        g5, b5 = _np(p["ln5"]["g"]), _np(p["ln5"]["b"])

        c = p["cross"]
        lin(f"b{bl}_cq", *_fold_ln(g1, b1, _np(c["q"]["W"]), _np(c["q"]["b"])))
        lin(f"b{bl}_ck", *_fold_ln(g2, b2, _np(c["k"][0]["W"]), _np(c["k"][0]["b"])))
        lin(f"b{bl}_cv", *_fold_ln(g2, b2, _np(c["v"][0]["W"]), _np(c["v"][0]["b"])))
        lin(f"b{bl}_cp", _np(c["proj"]["W"]), _np(c["proj"]["b"]))

        s = p["self"]
        lin(f"b{bl}_sq", *_fold_ln(g4, b4, _np(s["q"]["W"]), _np(s["q"]["b"])))
        lin(f"b{bl}_sk", *_fold_ln(g4, b4, _np(s["k"][0]["W"]), _np(s["k"][0]["b"])))
        lin(f"b{bl}_sv", *_fold_ln(g4, b4, _np(s["v"][0]["W"]), _np(s["v"][0]["b"])))
        lin(f"b{bl}_sp", _np(s["proj"]["W"]), _np(s["proj"]["b"]))

        for tag, (gg, bb), fp in (("f1", (g3, b3), p["mlp1"]),
                                  ("f2", (g5, b5), p["mlp2"])):
            W1, bf1 = _fold_ln(gg, bb, _np(fp["fc1"]["W"]), _np(fp["fc1"]["b"]))
            out[f"b{bl}_{tag}_w1"] = W1
            out[f"b{bl}_{tag}_b1"] = bf1.reshape(NINNER // 128, 128).T.copy()
            lin(f"b{bl}_{tag}_fc2", _np(fp["fc2"]["W"]), _np(fp["fc2"]["b"]))

    out["ident"] = np.eye(128, dtype=np.float32)
    out["bdzero"] = np.zeros((128, H + 2 * NHEAD), dtype=np.float32)
    out["ones_row"] = np.ones((1, 128), dtype=np.float32)
    zsel = np.zeros((H, NHEAD), dtype=np.float32)
    for h in range(NHEAD):
        zsel[h * DH:(h + 1) * DH, h] = 1.0
    out["zsel"] = zsel
    return out


# ----------------------------------------------------------------------------
# device program
# ----------------------------------------------------------------------------

class Prog:
    def __init__(self, nc, tc, nt, nzt, nblocks, use_cc):
        self.nc, self.tc = nc, tc
        self.nt, self.nzt, self.nblocks, self.use_cc = nt, nzt, nblocks, use_cc
        self.input_names = []

    def dram_in(self, name, shape, dt):
        ap = self.nc.dram_tensor(name, shape, dt, kind="ExternalInput").ap()
        self.input_names.append(name)
        return ap

    def build(self):
        nc, tc = self.nc, self.tc
        nt, nzt = self.nt, self.nzt

        self.d_x = self.dram_in("x_in", (nt * 128, 3), F32)
        self.d_y0 = self.dram_in("y0_in", (nzt * 128, 3), F32)
        self.d_out = nc.dram_tensor("y_out", (nt * 128, 3), F32,
                                    kind="ExternalOutput").ap()

        # weight dram tensors
        wnames = []
        for pre in ("tr", "br", "out"):
            for part in ("pre", "m0", "m1", "post"):
                wnames += [f"{pre}_{part}_w", f"{pre}_{part}_b"]
        for bl in range(self.nblocks):
            for t in ("cq", "ck", "cv", "cp", "sq", "sk", "sv", "sp"):
                wnames += [f"b{bl}_{t}_w", f"b{bl}_{t}_b"]
            for t in ("f1", "f2"):
                wnames += [f"b{bl}_{t}_w1", f"b{bl}_{t}_b1",
                           f"b{bl}_{t}_fc2_w", f"b{bl}_{t}_fc2_b"]
        wnames += ["ident", "ones_row", "zsel", "bdzero"]

        def wshape(n):
            if n == "ident":
                return (128, 128)
            if n == "ones_row":
                return (1, 128)
            if n == "zsel":
                return (H, NHEAD)
            if n == "bdzero":
                return (128, BDC)
            if n.endswith("_w1"):
                return (H, NINNER)
            if n.endswith("_b1"):
                return (128, NINNER // 128)
            if n.endswith("fc2_w"):
                return (NINNER, H)
            if n in ("tr_pre_w", "br_pre_w"):
                return (3, H)
            if n == "out_post_w":
                return (H, 4)
            if n == "out_post_b":
                return (1, 4)
            if n.endswith("_b"):
                return (1, H)
            return (H, H)

        self.d_w = {n: self.dram_in(n, wshape(n), F32 if n.endswith("_b1") else F32R)
                    for n in wnames}

        st = contextlib.ExitStack()
        self.res = st.enter_context(tc.tile_pool(name="res", bufs=1))
        self.wp = st.enter_context(tc.tile_pool(name="wp", bufs=2))
        self.wk = st.enter_context(tc.tile_pool(name="wkonst", bufs=1))
        self.work = st.enter_context(tc.tile_pool(name="work", bufs=3))
        self.small = st.enter_context(tc.tile_pool(name="small", bufs=6))
        self.ps_mm = st.enter_context(tc.tile_pool(name="psmm", bufs=3, space="PSUM"))
        self.ps_tp = st.enter_context(tc.tile_pool(name="pstp", bufs=3, space="PSUM"))
        self.ps_ctx = st.enter_context(tc.tile_pool(name="psctx", bufs=1, space="PSUM"))
        self.dram = st.enter_context(tc.tile_pool(name="dramp", bufs=1, space="DRAM"))

        # constants
        self.ident = self.wk.tile([128, 128], F32R, tag="ident")
        nc.sync.dma_start(self.ident[:], self.d_w["ident"])
        self.ones = self.wk.tile([1, 128], F32R, tag="ones")
        nc.sync.dma_start(self.ones[:], self.d_w["ones_row"])
        self.zsel = self.wk.tile([128, 2 * NHEAD], F32R, tag="zsel")
        nc.sync.dma_start(
            self.zsel[:].rearrange("p (k n) -> p k n", k=2),
            self.d_w["zsel"].rearrange("(k p) n -> p k n", p=128))
        self.bdzero = self.wk.tile([128, BDC], F32R, tag="bdzero")
        nc.sync.dma_start(self.bdzero[:], self.d_w["bdzero"])

        # residents
        self.x_res = self.res.tile([128, nt * H], F32, tag="x_res")
        self.z0T = self.res.tile([128, nzt * H], F32R, tag="z0T")
        self.xin = self.res.tile([128, nt * 3], F32, tag="xin")
        self.yout = self.res.tile([128, nt * 4], F32, tag="yout")
        self.yin = self.res.tile([128, nzt * 3], F32, tag="yin")
        nc.sync.dma_start(
            self.xin[:].rearrange("p (i c) -> p i c", c=3),
            self.d_x.rearrange("(i p) c -> p i c", p=128))
        nc.sync.dma_start(
            self.yin[:].rearrange("p (i c) -> p i c", c=3),
            self.d_y0.rearrange("(i p) c -> p i c", p=128))

        self.q_dram = self.dram.tile([128, nt * H], F32R, tag="q_dram")
        if self.use_cc:
            self.ar_in = self.dram.tile([128, 2 * AUX], F32, tag="ar_in")
            self.ar_out = self.dram.tile([128, 2 * AUX], F32, tag="ar_out")

        # ---- program ----
        # branch first: z scratch borrows the (not yet written) x_res cols
        self.branch_pass()
        self.mlp_pass(self.xin, nt, "tr", dst=self.x_res)
        for bl in range(self.nblocks):
            bd_c = self.cross_ctx(bl)
            self.attn_qside(bl, "c", bd_c)
            self.ffn(bl, "f1")
            self.self_attn(bl)
            self.ffn(bl, "f2")
        self.mlp_pass(self.x_res, nt, "out", dst=self.yout)
        nc.sync.dma_start(
            self.d_out.rearrange("(i p) c -> p i c", p=128),
            self.yout[:].rearrange("p (i c) -> p i c", c=4)[:, :, 0:3])
        st.close()

    # ---- primitives ----------------------------------------------------
    def load_w(self, name, tag):
        """[fin, fout] dram -> sbuf tile [P, kt*fout] (k-major col blocks)."""
        nc = self.nc
        d = self.d_w[name]
        fin, fout = d.shape
        if fin > 128:
            kt = fin // 128
            t = self.wp.tile([128, kt * fout], F32R, tag=tag,
                 bufs=1 if tag in ("w1", "wf2") else None)
            nc.sync.dma_start(
                t[:].rearrange("p (k n) -> p k n", k=kt),
                d.rearrange("(k p) n -> p k n", p=128))
        else:
            kt = 1
            t = self.wp.tile([fin, fout], F32R, tag=tag)
            nc.sync.dma_start(t[:], d)
        return t, kt, fout

    def load_brow(self, name):
        nc = self.nc
        d = self.d_w[name]
        t = self.small.tile([1, H], F32R, tag="brow")
        nc.sync.dma_start(t[0:1, 0:d.shape[1]], d)
        return t

    def transpose(self, src_ap, n_ch, dst_ap=None, in_f32=False, tag="Tt"):
        """[128, n_ch] sbuf -> transposed [128, n_ch] (k-major 128-blocks)."""
        nc = self.nc
        kt = n_ch // 128
        ps = self.ps_tp.tile([128, 256], F32 if in_f32 else F32R, tag="tp")
        ident = self.ident[:].bitcast(F32) if in_f32 else self.ident[:]
        for k in range(kt):
            nc.tensor.matmul(ps[:, k * 128:(k + 1) * 128],
                             src_ap[:, k * 128:(k + 1) * 128],
                             ident, is_transpose=True)
        if dst_ap is None:
            t = self.work.tile([128, n_ch], F32R, tag=tag)
            dst_ap = t[:]
        else:
            t = None
        nc.scalar.copy(dst_ap, ps[:, 0:n_ch])
        return t

    def linear(self, lhsT_aps, rhs_aps, n, brow=None, psum=None, tag="mm"):
        nc = self.nc
        if psum is None:
            psum = self.ps_mm.tile([128, AUX + NHEAD], F32, tag=tag)
        o = psum[:, 0:n]
        first = True
        if brow is not None:
            nc.tensor.matmul(o, self.ones[:], brow[0:1, 0:n], start=True,
                             stop=False)
            first = False
        last = len(lhsT_aps) - 1
        for k, (lt, rt) in enumerate(zip(lhsT_aps, rhs_aps)):
            nc.tensor.matmul(o, lt, rt, start=first, stop=(k == last))
            first = False
        return psum

    def wslices(self, W, kt, n):
        return [W[:, k * n:(k + 1) * n] for k in range(kt)]

    def tslices(self, xT, kt=2):
        return [xT[:, k * 128:(k + 1) * 128] for k in range(kt)]

    def ln_stats(self, x_slice_3d, ntile):
        nc = self.nc
        st = self.small.tile([128, ntile * 6], F32, tag="st")
        mv = self.small.tile([128, ntile * 2], F32, tag="mv")
        for t in range(ntile):
            nc.vector.bn_stats(st[:, 6 * t:6 * t + 6], x_slice_3d[:, t])
            nc.vector.bn_aggr(mv[:, 2 * t:2 * t + 2], st[:, 6 * t:6 * t + 6])
        ve = self.small.tile([128, ntile], F32, tag="ve")
        mv3 = mv[:].rearrange("p (t s) -> p t s", s=2)
        nc.vector.tensor_scalar(ve[:].rearrange("p (t s) -> p t s", s=1),
                                mv3[:, :, 1:2], EPS, None, ALU.add)
        nc.scalar.activation(ve[:], ve[:], AF.Sqrt)
        nc.vector.reciprocal(ve[:], ve[:])
        return mv, ve

    def ln_norm(self, x_slice, mv, ve, t):
        nc = self.nc
        xh = self.work.tile([128, H], F32R, tag="xhat")
        nc.vector.tensor_scalar(xh[:], x_slice, mv[:, 2 * t:2 * t + 1],
                                ve[:, t:t + 1], ALU.subtract, ALU.mult)
        return xh

    # ---- phases --------------------------------------------------------
    def mlp_pass(self, src, ntiles, pre, dst):
        """dst tile cols = ntiles*fout. src: [128, ntiles*3] or x_res."""
        nc = self.nc
        in3 = pre != "out"
        Wp, ktp, foutp = self.load_w(pre + "_pre_w", tag="wA")
        bp = self.load_brow(pre + "_pre_b")
        W0, kt0, _ = self.load_w(pre + "_m0_w", tag="wB")
        b0 = self.load_brow(pre + "_m0_b")
        W1, kt1, _ = self.load_w(pre + "_m1_w", tag="wC")
        b1 = self.load_brow(pre + "_m1_b")
        Wq, ktq, foutq = self.load_w(pre + "_post_w", tag="wD")
        bq = self.load_brow(pre + "_post_b")

        for i in range(ntiles):
            if in3:
                ps = self.ps_tp.tile([128, 256], F32, tag="tp")
                nc.tensor.matmul(ps[0:3, 0:128], src[:, i * 3:(i + 1) * 3],
                                 self.ident[:].bitcast(F32), is_transpose=True)
                xT = self.work.tile([3, 128], F32R, tag="Tin3", bufs=2)
                nc.scalar.copy(xT[:], ps[0:3, 0:128])
                lhs = [xT[:]]
            else:
                xT = self.transpose(src[:, i * H:(i + 1) * H], H, in_f32=True, tag='xT')
                lhs = self.tslices(xT[:])
            p = self.linear(lhs, self.wslices(Wp, ktp, foutp), foutp, brow=bp)
            g1 = self.work.tile([128, H], F32R, tag="mlpg", bufs=2)
            nc.scalar.activation(g1[:], p[:, 0:H], AF.Gelu)
            cur = g1
            for W, bb, kt in ((W0, b0, kt0), (W1, b1, kt1)):
                cT = self.transpose(cur[:], H, tag='cT')
                p = self.linear(self.tslices(cT[:]), self.wslices(W, kt, H), H,
                                brow=bb)
                nxt = self.work.tile([128, H], F32R, tag="mlpg", bufs=2)
                nc.scalar.activation(nxt[:], p[:, 0:H], AF.Gelu)
                nc.vector.tensor_tensor(nxt[:], nxt[:], cur[:], ALU.add)
                cur = nxt
            cT = self.transpose(cur[:], H, tag='cT')
            p = self.linear(self.tslices(cT[:]), self.wslices(Wq, ktq, foutq),
                            foutq, brow=bq)
            nc.scalar.copy(dst[:, i * foutq:(i + 1) * foutq], p[:, 0:foutq])

    def branch_pass(self):
        nc = self.nc
        z_tmp = self.x_res  # borrow before trunk writes it
        self.mlp_pass(self.yin, self.nzt, "br", dst=z_tmp)
        for i0 in range(0, self.nzt, 2):
            ntile = min(2, self.nzt - i0)
            x3 = z_tmp[:, i0 * H:(i0 + ntile) * H].rearrange(
                "p (t c) -> p t c", c=H)
            mv, ve = self.ln_stats(x3, ntile)
            for t in range(ntile):
                i = i0 + t
                zh = self.ln_norm(z_tmp[:, i * H:(i + 1) * H], mv, ve, t)
                self.transpose(zh[:], H, dst_ap=self.z0T[:, i * H:(i + 1) * H])

    def kv_ctx(self, lhsT_aps, Wk, bk, Wv, bv, ctx, first, last):
        """exp-k / Zk-folded-v projections of one tile; rank-update ctx."""
        nc = self.nc
        kp = self.linear(lhsT_aps, self.wslices(Wk, 2, H), H, brow=bk)
        ek = self.work.tile([128, H], F32R, tag="ek")
        nc.scalar.activation(ek[:], kp[:, 0:H], AF.Exp)
        ekT = self.transpose(ek[:], H, tag='ekT')
        zp = self.linear(self.tslices(ekT[:]), self.wslices(self.zsel, 2, NHEAD),
                         NHEAD)
        va = self.work.tile([128, AUX], F32R, tag="vaug")
        with nc.allow_low_precision(reason="f32r rounding of 1/Zk is fine"):
            nc.vector.reciprocal(va[:, H:AUX], zp[:, 0:NHEAD])
        vp = self.linear(lhsT_aps, self.wslices(Wv, 2, H), H, brow=bv)
        nc.vector.tensor_tensor(
            va[:, 0:H].rearrange("p (g c) -> p g c", g=NHEAD),
            vp[:, 0:H].rearrange("p (g c) -> p g c", g=NHEAD),
            va[:, H:AUX].to_broadcast((128, NHEAD, DH)),
            ALU.mult)
        for m in range(2):
            nc.tensor.matmul(ctx[m][:, 0:AUX], ek[:, m * 128:(m + 1) * 128],
                             va[:, 0:AUX], start=first, stop=last)

    def build_bd(self, src, ncols, z_ones):
        """src(k) -> [128, AUX] AP. Returns bd tiles [2][128, ncols]."""
        nc = self.nc
        bd = []
        for k in range(2):
            t = self.work.tile([128, ncols], F32R, tag="bd", bufs=2)
            nc.vector.tensor_copy(t[:], self.bdzero[:, 0:ncols])
            s = src(k)
            for j in range(4):
                h = 4 * k + j
                r0, r1 = 32 * j, 32 * j + 32
                nc.vector.tensor_copy(t[r0:r1, h * DH:(h + 1) * DH],
                                      s[r0:r1, h * DH:(h + 1) * DH])
                nc.vector.tensor_copy(t[r0:r1, H + h:H + h + 1],
                                      s[r0:r1, H + h:H + h + 1])
            if z_ones:
                nc.vector.tensor_copy(t[:, H + NHEAD:H + 2 * NHEAD],
                                      self.zsel[:, k * NHEAD:(k + 1) * NHEAD])
            bd.append(t)
        return bd

    def cross_ctx(self, bl):
        nc = self.nc
        Wk, _, _ = self.load_w(f"b{bl}_ck_w", tag="wB")
        bk = self.load_brow(f"b{bl}_ck_b")
        Wv, _, _ = self.load_w(f"b{bl}_cv_w", tag="wC")
        bv = self.load_brow(f"b{bl}_cv_b")
        ctx = [self.ps_ctx.tile([128, AUX], F32, tag=f"ctx{m}", name=f"ctx{m}")
               for m in range(2)]
        for i in range(self.nzt):
            self.kv_ctx(self.tslices(self.z0T[:, i * H:(i + 1) * H]),
                        Wk, bk, Wv, bv, ctx, i == 0, i == self.nzt - 1)
        return self.build_bd(lambda k: ctx[k][:, 0:AUX], BDC, z_ones=True)

    def attn_qside(self, bl, which, bd):
        """Fused q-side attention + proj + residual (cross: which='c')."""
        nc = self.nc
        Wq, _, _ = self.load_w(f"b{bl}_{which}q_w", tag="wA")
        bq = self.load_brow(f"b{bl}_{which}q_b")
        Wp, _, _ = self.load_w(f"b{bl}_{which}p_w", tag="wD")
        bp = self.load_brow(f"b{bl}_{which}p_b")
        for i0 in range(0, self.nt, 2):
            x3 = self.x_res[:, i0 * H:(i0 + 2) * H].rearrange(
                "p (t c) -> p t c", c=H)
            mv, ve = self.ln_stats(x3, 2)
            for t in range(2):
                i = i0 + t
                xh = self.ln_norm(self.x_res[:, i * H:(i + 1) * H], mv, ve, t)
                xT = self.transpose(xh[:], H, tag='xT')
                qp = self.linear(self.tslices(xT[:]), self.wslices(Wq, 2, H),
                                 H, brow=bq)
                qe = self.work.tile([128, H], F32R, tag="qe")
                nc.scalar.activation(qe[:], qp[:, 0:H], AF.Exp)
                qT = self.transpose(qe[:], H, tag='qT')
                npp = self.linear(self.tslices(qT[:]),
                                  [bd[0][:], bd[1][:]], BDC)
                cb = self.small.tile([128, 2 * NHEAD], F32, tag="cb")
                nc.vector.reciprocal(cb[:], npp[:, H:H + 2 * NHEAD])
                t1 = self.work.tile([128, H], F32R, tag="t1")
                nc.vector.tensor_tensor(
                    t1[:].rearrange("p (g c) -> p g c", g=NHEAD),
                    npp[:, 0:H].rearrange("p (g c) -> p g c", g=NHEAD),
                    cb[:, 0:NHEAD].to_broadcast((128, NHEAD, DH)), ALU.mult)
                qx = self.work.tile([128, H], F32R, tag="qx", bufs=2)
                nc.vector.tensor_tensor(
                    qx[:].rearrange("p (g c) -> p g c", g=NHEAD),
                    qe[:].rearrange("p (g c) -> p g c", g=NHEAD),
                    cb[:, NHEAD:2 * NHEAD].to_broadcast((128, NHEAD, DH)),
                    ALU.mult)
                nc.vector.tensor_tensor(t1[:], t1[:], qx[:], ALU.add)
                self.attn_tail(t1, Wp, bp, i)

    def attn_tail(self, attn, Wp, bp, i):
        nc = self.nc
        aT = self.transpose(attn[:], H, tag='aT')
        pp = self.linear(self.tslices(aT[:]), self.wslices(Wp, 2, H), H,
                         brow=bp)
        xs = self.x_res[:, i * H:(i + 1) * H]
        nc.vector.tensor_tensor(xs, xs, pp[:, 0:H], ALU.add)

    def self_attn(self, bl):
        nc = self.nc
        Wq, _, _ = self.load_w(f"b{bl}_sq_w", tag="wA")
        bq = self.load_brow(f"b{bl}_sq_b")
        Wk, _, _ = self.load_w(f"b{bl}_sk_w", tag="wB")
        bk = self.load_brow(f"b{bl}_sk_b")
        Wv, _, _ = self.load_w(f"b{bl}_sv_w", tag="wC")
        bv = self.load_brow(f"b{bl}_sv_b")
        ctx = [self.ps_ctx.tile([128, AUX], F32, tag=f"ctx{m}", name=f"ctx{m}")
               for m in range(2)]
        # pass 1: q-> normalize -> q_dram; k/v -> ctx
        for i0 in range(0, self.nt, 2):
            x3 = self.x_res[:, i0 * H:(i0 + 2) * H].rearrange(
                "p (t c) -> p t c", c=H)
            mv, ve = self.ln_stats(x3, 2)
            for t in range(2):
                i = i0 + t
                xh = self.ln_norm(self.x_res[:, i * H:(i + 1) * H], mv, ve, t)
                xT = self.transpose(xh[:], H)
                lhs = self.tslices(xT[:])
                qp = self.linear(lhs, self.wslices(Wq, 2, H), H, brow=bq)
                qe = self.work.tile([128, H], F32R, tag="qe")
                nc.scalar.activation(qe[:], qp[:, 0:H], AF.Exp)
                qT = self.transpose(qe[:], H, tag='qT')
                zq = self.linear(self.tslices(qT[:]),
                                 self.wslices(self.zsel, 2, NHEAD), NHEAD)
                zi = self.small.tile([128, NHEAD], F32, tag="zi")
                nc.vector.reciprocal(zi[:], zq[:, 0:NHEAD])
                qn = self.work.tile([128, H], F32R, tag="qn")
                nc.vector.tensor_tensor(
                    qn[:].rearrange("p (g c) -> p g c", g=NHEAD),
                    qe[:].rearrange("p (g c) -> p g c", g=NHEAD),
                    zi[:].to_broadcast((128, NHEAD, DH)), ALU.mult)
                nc.sync.dma_start(self.q_dram[:, i * H:(i + 1) * H], qn[:])
                self.kv_ctx(lhs, Wk, bk, Wv, bv, ctx, i == 0, i == self.nt - 1)

        # combine pair ctx
        if self.use_cc:
            stage = self.work.tile([128, 2 * AUX], F32, tag="stage", bufs=1)
            for k in range(2):
                nc.scalar.copy(stage[:, k * AUX:(k + 1) * AUX], ctx[k][:, 0:AUX])
            nc.sync.dma_start(self.ar_in[:], stage[:])
            nc.gpsimd.collective_compute(
                "AllReduce", ALU.add,
                replica_groups=[[0, 1], [2, 3], [4, 5], [6, 7]],
                ins=[self.ar_in.opt()], outs=[self.ar_out.opt()])
            stage_r = self.work.tile([128, 2 * AUX], F32, tag="stager", bufs=1)
            nc.sync.dma_start(stage_r[:], self.ar_out[:])
            bd = self.build_bd(lambda k: stage_r[:, k * AUX:(k + 1) * AUX],
                               AUX, z_ones=False)
        else:
            bd = self.build_bd(lambda k: ctx[k][:, 0:AUX], AUX, z_ones=False)

        Wp, _, _ = self.load_w(f"b{bl}_sp_w", tag="wD")
        bp = self.load_brow(f"b{bl}_sp_b")
        # pass 2
        for i in range(self.nt):
            qn = self.work.tile([128, H], F32R, tag="qn")
            nc.sync.dma_start(qn[:], self.q_dram[:, i * H:(i + 1) * H])
            qT = self.transpose(qn[:], H, tag='qT')
            npp = self.linear(self.tslices(qT[:]), [bd[0][:], bd[1][:]], AUX)
            cb = self.small.tile([128, NHEAD], F32, tag="cb2")
            nc.vector.reciprocal(cb[:], npp[:, H:AUX])
            t1 = self.work.tile([128, H], F32R, tag="t1")
            nc.vector.tensor_tensor(
                t1[:].rearrange("p (g c) -> p g c", g=NHEAD),
                npp[:, 0:H].rearrange("p (g c) -> p g c", g=NHEAD),
                cb[:].to_broadcast((128, NHEAD, DH)), ALU.mult)
            nc.vector.tensor_tensor(t1[:], t1[:], qn[:], ALU.add)
            self.attn_tail(t1, Wp, bp, i)

    def ffn(self, bl, tag):
        nc = self.nc
        W1, _, _ = self.load_w(f"b{bl}_{tag}_w1", tag="w1")
        b1 = self.wp.tile([128, NINNER // 128], F32, tag="b1c")
        nc.sync.dma_start(b1[:], self.d_w[f"b{bl}_{tag}_b1"])
        W2, _, _ = self.load_w(f"b{bl}_{tag}_fc2_w", tag="wf2")
        b2 = self.load_brow(f"b{bl}_{tag}_fc2_b")
        MT = NINNER // 128
        for i0 in range(0, self.nt, 2):
            x3 = self.x_res[:, i0 * H:(i0 + 2) * H].rearrange(
                "p (t c) -> p t c", c=H)
            mv, ve = self.ln_stats(x3, 2)
            xTc = self.work.tile([128, 2 * H], F32R, tag="xTc", bufs=2)
            for t in range(2):
                xh = self.ln_norm(self.x_res[:, (i0 + t) * H:(i0 + t + 1) * H],
                                  mv, ve, t)
                self.transpose(xh[:], H, dst_ap=xTc[:, t * H:(t + 1) * H])
            # fc1 (mapping A): h1T[m] = W1[:,m].T @ xhatT ; gelu w/ bias
            h1 = self.work.tile([128, MT * 256], F32R, tag="h1c", bufs=2)
            x3c = xTc[:].rearrange("p (t c) -> p t c", c=H)
            for m in range(MT):
                pm = self.ps_mm.tile([128, AUX + NHEAD], F32, tag="mm")
                for k in range(2):
                    nc.tensor.matmul(
                        pm[:, 0:256],
                        W1[:, k * NINNER + m * 128:k * NINNER + (m + 1) * 128],
                        x3c[:, :, k * 128:(k + 1) * 128],
                        start=(k == 0), stop=(k == 1))
                nc.scalar.activation(h1[:, m * 256:(m + 1) * 256], pm[:, 0:256],
                                     AF.Gelu, bias=b1[:, m:m + 1])
            # fc2 (mapping B): per tile
            for t in range(2):
                lhs = [h1[:, k * 256 + t * 128:k * 256 + t * 128 + 128]
                       for k in range(MT)]
                pf = self.linear(lhs, self.wslices(W2, MT, H), H, brow=b2)
                xs = self.x_res[:, (i0 + t) * H:(i0 + t + 1) * H]
                nc.vector.tensor_tensor(xs, xs, pf[:, 0:H], ALU.add)


# ----------------------------------------------------------------------------
# build + run
# ----------------------------------------------------------------------------

_CACHE = {}


def build_nc(nt=NT_FULL, nzt=NZT_FULL, nblocks=NB_FULL, use_cc=True,
             num_devices=NCORES):
    nc = bacc.Bacc("TRN2", target_bir_lowering=False, debug=False,
                   num_devices=num_devices)
    with tile.TileContext(nc) as tc:
        prog = Prog(nc, tc, nt, nzt, nblocks, use_cc)
        prog.build()
    nc.compile()
    return nc, prog


def _shard_inputs(x, y0, warrs, nt, nzt, ncores):
    """Returns per-core input dicts."""
    x = np.asarray(x, np.float32)
    y0 = np.asarray(y0, np.float32)
    maps = []
    half = nt * 128
    for c in range(ncores):
        b, h = c // 2, c % 2
        m = dict(warrs)
        m["x_in"] = np.ascontiguousarray(x[b, h * half:(h + 1) * half, :])
        m["y0_in"] = np.ascontiguousarray(y0[b, :nzt * 128, :])
        maps.append(m)
    return maps


class Runner:
    """Persistent PJRT executor for one built Bass program (axon path).
    Mirrors bass2jax.run_bass_via_pjrt but keeps the jitted callable so
    repeated executions don't recompile."""

    def __init__(self, nc, n_cores=NCORES):
        import jax
        from jax.sharding import Mesh, PartitionSpec
        from jax.experimental.shard_map import shard_map
        import concourse.mybir as mybir_
        from concourse import bass2jax

        bass2jax.install_neuronx_cc_hook()
        self.nc = nc
        self.n_cores = n_cores
        partition_name = (nc.partition_id_tensor.name
                          if nc.partition_id_tensor else None)
        in_names, out_names, out_avals, zero_outs = [], [], [], []
        for alloc in nc.m.functions[0].allocations:
            if not isinstance(alloc, mybir_.MemoryLocationSet):
                continue
            name = alloc.memorylocations[0].name
            if alloc.kind == "ExternalInput":
                if name == partition_name:
                    continue
                in_names.append(name)
            elif alloc.kind == "ExternalOutput":
                shape = tuple(alloc.tensor_shape)
                dtype = mybir_.dt.np(alloc.dtype)
                out_names.append(name)
                out_avals.append(jax.core.ShapedArray(shape, dtype))
                zero_outs.append(np.zeros(shape, dtype))
        self.in_names, self.out_names = in_names, out_names
        self.out_avals, self.zero_outs = out_avals, zero_outs
        n_params, n_outs = len(in_names), len(out_names)
        all_in = in_names + out_names
        if partition_name is not None:
            all_in = all_in + [partition_name]

        def _body(*args):
            operands = list(args)
            if partition_name is not None:
                operands.append(bass2jax.partition_id_tensor())
            outs = bass2jax._bass_exec_p.bind(
                *operands,
                out_avals=tuple(out_avals),
                in_names=tuple(all_in),
                out_names=tuple(out_names),
                lowering_input_output_aliases=(),
                sim_require_finite=True,
                sim_require_nnan=True,
                nc=nc,
            )
            return tuple(outs)

        devices = jax.devices()[:n_cores]
        mesh = Mesh(np.asarray(devices), ("core",))
        self.mesh = mesh
        donate = tuple(range(n_params, n_params + n_outs))
        self._fn = jax.jit(
            shard_map(_body, mesh=mesh,
                      in_specs=(PartitionSpec("core"),) * (n_params + n_outs),
                      out_specs=(PartitionSpec("core"),) * n_outs,
                      check_rep=False),
            donate_argnums=donate, keep_unused=True)

    def __call__(self, in_maps):
        concat_in = [
            np.concatenate([np.asarray(m[name]) for m in in_maps], axis=0)
            for name in self.in_names]
        concat_zeros = [
            np.zeros((self.n_cores * z.shape[0], *z.shape[1:]), z.dtype)
            for z in self.zero_outs]
        out_arrs = self._fn(*concat_in, *concat_zeros)
        return [
            {name: np.asarray(out_arrs[i]).reshape(
                self.n_cores, *self.out_avals[i].shape)[c]
             for i, name in enumerate(self.out_names)}
            for c in range(self.n_cores)]

    def run_async(self, in_maps):
        """Dispatch without converting outputs (returns jax arrays)."""
        concat_in = [
            np.concatenate([np.asarray(m[name]) for m in in_maps], axis=0)
            for name in self.in_names]
        concat_zeros = [
            np.zeros((self.n_cores * z.shape[0], *z.shape[1:]), z.dtype)
            for z in self.zero_outs]
        return self._fn(*concat_in, *concat_zeros)

    def stage(self, in_maps):
        """Transfer inputs to devices once; returns committed jax arrays."""
        import jax
        from jax.sharding import NamedSharding, PartitionSpec
        sh = NamedSharding(self.mesh, PartitionSpec("core"))
        staged = []
        for name in self.in_names:
            a = np.concatenate([np.asarray(m[name]) for m in in_maps], axis=0)
            staged.append(jax.device_put(a, sh))
        for s_ in staged:
            s_.block_until_ready()
        return staged

    def exec_staged(self, staged):
        concat_zeros = [
            np.zeros((self.n_cores * z.shape[0], *z.shape[1:]), z.dtype)
            for z in self.zero_outs]
        return self._fn(*staged, *concat_zeros)


def run_spmd(nc, in_maps):
    import concourse.bass_utils as bass_utils
    res = bass_utils.run_bass_kernel_spmd(
        nc, in_maps, core_ids=list(range(len(in_maps))))
    return res.results


def get_runner():
    if "full" not in _CACHE:
        nc, prog = build_nc()
        _CACHE["full"] = (nc, prog)
        _CACHE["runner"] = Runner(nc)
    return _CACHE["runner"]


def kernel(x, y0, params):
    x = np.asarray(x, np.float32)
    y0 = np.asarray(y0, np.float32)
    runner = get_runner()
    warrs = build_weight_arrays(params, NB_FULL)
    in_maps = _shard_inputs(x, y0, warrs, NT_FULL, NZT_FULL, NCORES)
    results = runner(in_maps)
    out = np.empty((B, T1, 3), np.float32)
    half = NT_FULL * 128
    for c in range(NCORES):
        b, h = c // 2, c % 2
        out[b, h * half:(h + 1) * half, :] = results[c]["y_out"]
    return out
